# revision 2
# baseline (speedup 1.0000x reference)
"""Trainium2 Bass kernel for ExpanderLinear: out = x @ (W * mask).T

Shapes (hardcoded): x [8192, 4096] f32, weight [4096, 4096] f32,
mask [4096, 4096] f32 -> out [8192, 4096] f32.

Strategy: tensor-parallel over output features across 8 cores, all
operands pre-converted to bf16 on the host (tolerance is 2e-2; bf16
matmul error here is ~2e-3). Host marshalling:
  xT bf16 [4096, 8192], wmT = ((W*mask).T) column slice bf16
  [4096, 512] per core.
Each core computes outT_c = wmT_c.T @ xT as [512, 8192] via bf16
matmuls (1 cycle/row, same PE rate as f32r but half the DMA traffic
and zero DVE rounding work); the host transposes/concatenates.

Per-core device kernel:
  - wmT resident in SBUF as 8 eighth tiles [128, 4, 512] bf16 (4 MB).
  - per 512-col batch chunk of xT: 8 sub-DMAs [128, 4, 512] bf16,
    then 4 psum tiles [128 out, 512 b] each accumulate 32 matmuls
    (lhsT = wm slice [128 k, 128 out], rhs = x sub [128 k, 512 b]).
  - psum -> SBUF f32 copy -> DMA to outT.
PE does matmuls only; DMA (84 MB/core) runs at ~54% duty so the
kernel sits at the dense-GEMM compute roofline.
"""

import ml_dtypes
import numpy as np

import concourse.bass as bass
import concourse.mybir as mybir
import concourse.tile as tile
from concourse import bacc
from concourse.bass_utils import run_bass_kernel_spmd

P = 128
D_IN = 4096
D_OUT = 4096
BATCH = 8192
N_CORES = 8
O_PER_CORE = D_OUT // N_CORES  # 512
KC = D_IN // P  # 32 contraction chunks of 128
B_CHUNK = 512
N_BCHUNK = BATCH // B_CHUNK  # 16
OT = O_PER_CORE // P  # 4 output partition tiles
KG = 8  # k groups per batch chunk (sub-DMA granularity)
KCG = KC // KG  # 4 ks of 128 per group

F32 = mybir.dt.float32
BF16 = mybir.dt.bfloat16


def build_nc():
    nc = bacc.Bacc("TRN2", target_bir_lowering=False, debug=False, num_devices=N_CORES)

    xT_d = nc.dram_tensor("xT", [D_IN, BATCH], BF16, kind="ExternalInput")
    wmT_d = nc.dram_tensor("wmT", [D_IN, O_PER_CORE], BF16, kind="ExternalInput")
    outT_d = nc.dram_tensor("outT", [O_PER_CORE, BATCH], F32, kind="ExternalOutput")

    with tile.TileContext(nc) as tc:
        with (
            tc.tile_pool(name="persist", bufs=1) as persist,
            tc.tile_pool(name="xs", bufs=16) as xspool,
            tc.tile_pool(name="outp", bufs=8) as outp,
            tc.tile_pool(name="mpsum", bufs=8, space="PSUM") as mpsum,
        ):
            # --- resident weights: 8 eighth tiles [128, 4, 512] bf16,
            # interleaved with chunk 0's x loads so the first matmuls
            # start as soon as the first ~1 MB has landed ---
            wm_e = []

            def emit_wm_eighth(e):
                r_sl = slice(e * KCG * P, (e + 1) * KCG * P)
                wm = persist.tile([P, KCG, O_PER_CORE], BF16, name=f"wmT{e}")
                nc.sync.dma_start(
                    wm, wmT_d[r_sl, :].rearrange("(kc p) o -> p kc o", p=P)
                )
                wm_e.append(wm)

            def emit_x_sub(bc, g):
                xs = xspool.tile([P, KCG, B_CHUNK], BF16, tag="xs", name="xs")
                rows = slice(g * (D_IN // KG), (g + 1) * (D_IN // KG))
                cols = slice(bc * B_CHUNK, (bc + 1) * B_CHUNK)
                nc.sync.dma_start(
                    xs, xT_d[rows, cols].rearrange("(kc p) b -> p kc b", p=P)
                )
                return xs

            pending = []
            for e in range(KG):
                emit_wm_eighth(e)
                pending.append(emit_x_sub(0, e))

            def lhsT(ic, oc):
                return wm_e[ic // KCG][:, ic % KCG, oc * P : (oc + 1) * P]

            # --- main loop over batch chunks ---
            for bc in range(N_BCHUNK):
                x_subs = pending
                psums = [
                    mpsum.tile([P, B_CHUNK], F32, name=f"ps{oc}", tag="ps")
                    for oc in range(OT)
                ]
                last = bc == N_BCHUNK - 1
                if last:
                    # oc-major so each psum finishes early and its drain +
                    # output DMA overlap the remaining matmuls (short tail)
                    for oc in range(OT):
                        for g in range(KG):
                            for k in range(KCG):
                                ic = g * KCG + k
                                nc.tensor.matmul(
                                    psums[oc],
                                    lhsT(ic, oc),
                                    x_subs[g][:, k, :],
                                    start=(ic == 0),
                                    stop=(ic == KC - 1),
                                )
                        ob = outp.tile([P, B_CHUNK], F32)
                        nc.vector.tensor_copy(ob, psums[oc])
                        nc.sync.dma_start(
                            outT_d[
                                oc * P : (oc + 1) * P,
                                bc * B_CHUNK : (bc + 1) * B_CHUNK,
                            ],
                            ob,
                        )
                    continue
                for g in range(KG):
                    for k in range(KCG):
                        ic = g * KCG + k
                        for oc in range(OT):
                            nc.tensor.matmul(
                                psums[oc],
                                lhsT(ic, oc),
                                x_subs[g][:, k, :],
                                start=(ic == 0),
                                stop=(ic == KC - 1),
                            )
                if bc + 1 < N_BCHUNK:
                    pending = [emit_x_sub(bc + 1, g) for g in range(KG)]
                for oc in range(OT):
                    ob = outp.tile([P, B_CHUNK], F32)
                    nc.vector.tensor_copy(ob, psums[oc])
                    nc.sync.dma_start(
                        outT_d[
                            oc * P : (oc + 1) * P, bc * B_CHUNK : (bc + 1) * B_CHUNK
                        ],
                        ob,
                    )

    nc.compile()
    return nc


_NC_CACHE = None


def _shard_inputs(x, weight, mask):
    """Host-side marshalling: mask-multiply, transpose, bf16, per-core slice."""
    x = np.asarray(x, dtype=np.float32)
    weight = np.asarray(weight, dtype=np.float32)
    mask = np.asarray(mask, dtype=np.float32)
    xT = np.ascontiguousarray(x.T).astype(ml_dtypes.bfloat16)
    wmT = (weight * mask).T  # [D_IN, D_OUT]
    in_maps = []
    for c in range(N_CORES):
        sl = slice(c * O_PER_CORE, (c + 1) * O_PER_CORE)
        in_maps.append(
            {
                "xT": xT,
                "wmT": np.ascontiguousarray(wmT[:, sl]).astype(ml_dtypes.bfloat16),
            }
        )
    return in_maps


def kernel(x, weight, mask):
    global _NC_CACHE
    if _NC_CACHE is None:
        _NC_CACHE = build_nc()
    nc = _NC_CACHE

    in_maps = _shard_inputs(x, weight, mask)
    res = run_bass_kernel_spmd(nc, in_maps, core_ids=list(range(N_CORES)))

    out = np.empty((BATCH, D_OUT), dtype=np.float32)
    for c in range(N_CORES):
        sl = slice(c * O_PER_CORE, (c + 1) * O_PER_CORE)
        out[:, sl] = res.results[c]["outT"].T
    return out


# revision 3
# speedup vs baseline: 1.1726x; 1.1726x over previous
"""Trainium2 Bass kernel for ExpanderLinear: out = x @ (W * mask).T

Shapes (hardcoded): x [8192, 4096] f32, weight [4096, 4096] f32,
mask [4096, 4096] f32 -> out [8192, 4096] f32.

Strategy: tensor-parallel over output features across 8 cores, all
operands pre-converted to bf16 on the host (tolerance is 2e-2; bf16
matmul error here is ~2.5e-3). Host marshalling:
  xT bf16 [4096, 8192], wmT = ((W*mask).T) column slice bf16
  [4096, 512] per core.
Each core computes outT_c = wmT_c.T @ xT as [512, 8192] via bf16
matmuls (1 cycle/row, same PE rate as f32r but half the DMA traffic
and zero DVE rounding work); the host transposes/concatenates.

Per-core device kernel:
  - wmT resident in SBUF as 32 single tiles [128, 512] bf16 (4 MB),
    interleaved with chunk 0's x singles so the first matmul starts
    ~8 us in (after the ~5 us engine preamble + first 256 KB of DMA).
  - per 512-col batch chunk of xT: 8 sub-DMAs [128, 4, 512] bf16,
    then 4 psum tiles [128 out, 512 b] each accumulate 32 matmuls
    (lhsT = wm slice [128 k, 128 out], rhs = x sub [128 k, 512 b]).
  - last chunk runs as 2 half-width (256 col) chunks, oc-major, so
    the final psum drain + output DMA tail is halved.
  - psum -> SBUF f32 copy -> DMA to outT.
PE does matmuls only; DMA (84 MB/core) runs at ~54% duty so the
kernel sits at the dense-GEMM compute roofline (~443 us span at
2.4 GHz; chip power states can derate the PE to ~2.0 GHz run-to-run).
"""

import ml_dtypes
import numpy as np

import concourse.bass as bass
import concourse.mybir as mybir
import concourse.tile as tile
from concourse import bacc
from concourse.bass_utils import run_bass_kernel_spmd

P = 128
D_IN = 4096
D_OUT = 4096
BATCH = 8192
N_CORES = 8
O_PER_CORE = D_OUT // N_CORES  # 512
KC = D_IN // P  # 32 contraction chunks of 128
B_CHUNK = 512
N_BCHUNK = BATCH // B_CHUNK  # 16
OT = O_PER_CORE // P  # 4 output partition tiles
KG = 8  # k groups per batch chunk (sub-DMA granularity)
KCG = KC // KG  # 4 ks of 128 per group

F32 = mybir.dt.float32
BF16 = mybir.dt.bfloat16


def build_nc():
    nc = bacc.Bacc("TRN2", target_bir_lowering=False, debug=False, num_devices=N_CORES)

    xT_d = nc.dram_tensor("xT", [D_IN, BATCH], BF16, kind="ExternalInput")
    wmT_d = nc.dram_tensor("wmT", [D_IN, O_PER_CORE], BF16, kind="ExternalInput")
    outT_d = nc.dram_tensor("outT", [O_PER_CORE, BATCH], F32, kind="ExternalOutput")

    with tile.TileContext(nc) as tc:
        with (
            tc.tile_pool(name="persist", bufs=1) as persist,
            tc.tile_pool(name="x0", bufs=KC) as x0pool,
            tc.tile_pool(name="xs", bufs=16) as xspool,
            tc.tile_pool(name="outp", bufs=8) as outp,
            tc.tile_pool(name="mpsum", bufs=8, space="PSUM") as mpsum,
        ):
            # --- prologue: weights as 32 singles [128, 512], finely
            # interleaved with chunk 0's x singles so MM(ic=0) only waits
            # for 256 KB of DMA (not 1 MB) ---
            wm_t = []
            x0_t = []
            for ic in range(KC):
                wm = persist.tile([P, O_PER_CORE], BF16, name=f"wmT{ic}")
                nc.sync.dma_start(wm, wmT_d[ic * P : (ic + 1) * P, :])
                wm_t.append(wm)
                x0 = x0pool.tile([P, B_CHUNK], BF16, tag="x0", name=f"x0_{ic}")
                nc.sync.dma_start(x0, xT_d[ic * P : (ic + 1) * P, 0:B_CHUNK])
                x0_t.append(x0)

            def lhsT(ic, oc):
                return wm_t[ic][:, oc * P : (oc + 1) * P]

            def emit_x_sub(bc, g):
                xs = xspool.tile([P, KCG, B_CHUNK], BF16, tag="xs", name="xs")
                rows = slice(g * (D_IN // KG), (g + 1) * (D_IN // KG))
                cols = slice(bc * B_CHUNK, (bc + 1) * B_CHUNK)
                nc.sync.dma_start(
                    xs, xT_d[rows, cols].rearrange("(kc p) b -> p kc b", p=P)
                )
                return xs

            def drain(psum, oc, bcol, ncol):
                ob = outp.tile([P, ncol], F32)
                nc.vector.tensor_copy(ob, psum)
                nc.sync.dma_start(
                    outT_d[oc * P : (oc + 1) * P, bcol : bcol + ncol], ob
                )

            # --- chunk 0: rhs from the 32 singles ---
            psums = [
                mpsum.tile([P, B_CHUNK], F32, name=f"ps{oc}", tag="ps")
                for oc in range(OT)
            ]
            for ic in range(KC):
                for oc in range(OT):
                    nc.tensor.matmul(
                        psums[oc],
                        lhsT(ic, oc),
                        x0_t[ic],
                        start=(ic == 0),
                        stop=(ic == KC - 1),
                    )
            pending = [emit_x_sub(1, g) for g in range(KG)]
            for oc in range(OT):
                drain(psums[oc], oc, 0, B_CHUNK)

            # --- chunks 1..14: full-width, k-major so all 4 psums
            # finish together; prefetch next chunk before draining ---
            for bc in range(1, N_BCHUNK - 1):
                x_subs = pending
                psums = [
                    mpsum.tile([P, B_CHUNK], F32, name=f"ps{oc}", tag="ps")
                    for oc in range(OT)
                ]
                for g in range(KG):
                    for k in range(KCG):
                        ic = g * KCG + k
                        for oc in range(OT):
                            nc.tensor.matmul(
                                psums[oc],
                                lhsT(ic, oc),
                                x_subs[g][:, k, :],
                                start=(ic == 0),
                                stop=(ic == KC - 1),
                            )
                if bc + 1 < N_BCHUNK:
                    pending = [emit_x_sub(bc + 1, g) for g in range(KG)]
                for oc in range(OT):
                    drain(psums[oc], oc, bc * B_CHUNK, B_CHUNK)

            # --- last chunk: two half-width (256 col) passes, oc-major,
            # so each psum finishes early and the final drain tail is
            # half a tile ---
            x_subs = pending
            HB = B_CHUNK // 2
            for h in range(2):
                for oc in range(OT):
                    ps = mpsum.tile([P, HB], F32, name=f"psl{h}{oc}", tag="ps")
                    for g in range(KG):
                        for k in range(KCG):
                            ic = g * KCG + k
                            nc.tensor.matmul(
                                ps,
                                lhsT(ic, oc),
                                x_subs[g][:, k, h * HB : (h + 1) * HB],
                                start=(ic == 0),
                                stop=(ic == KC - 1),
                            )
                    drain(ps, oc, (N_BCHUNK - 1) * B_CHUNK + h * HB, HB)

    nc.compile()
    return nc


_NC_CACHE = None


def _shard_inputs(x, weight, mask):
    """Host-side marshalling: mask-multiply, transpose, bf16, per-core slice."""
    x = np.asarray(x, dtype=np.float32)
    weight = np.asarray(weight, dtype=np.float32)
    mask = np.asarray(mask, dtype=np.float32)
    xT = np.ascontiguousarray(x.T).astype(ml_dtypes.bfloat16)
    wmT = (weight * mask).T  # [D_IN, D_OUT]
    in_maps = []
    for c in range(N_CORES):
        sl = slice(c * O_PER_CORE, (c + 1) * O_PER_CORE)
        in_maps.append(
            {
                "xT": xT,
                "wmT": np.ascontiguousarray(wmT[:, sl]).astype(ml_dtypes.bfloat16),
            }
        )
    return in_maps


def kernel(x, weight, mask):
    global _NC_CACHE
    if _NC_CACHE is None:
        _NC_CACHE = build_nc()
    nc = _NC_CACHE

    in_maps = _shard_inputs(x, weight, mask)
    res = run_bass_kernel_spmd(nc, in_maps, core_ids=list(range(N_CORES)))

    out = np.empty((BATCH, D_OUT), dtype=np.float32)
    for c in range(N_CORES):
        sl = slice(c * O_PER_CORE, (c + 1) * O_PER_CORE)
        out[:, sl] = res.results[c]["outT"].T
    return out


# revision 4
# speedup vs baseline: 1.2030x; 1.0259x over previous
"""Trainium2 Bass kernel for ExpanderLinear: out = x @ (W * mask).T

Shapes (hardcoded): x [8192, 4096] f32, weight [4096, 4096] f32,
mask [4096, 4096] f32 -> out [8192, 4096] f32.

Strategy: tensor-parallel over output features across 8 cores, all
operands pre-converted to bf16 on the host (tolerance is 2e-2; bf16
matmul error here is ~2.5e-3). Host marshalling:
  xT bf16 [4096, 8192], wmT = ((W*mask).T) column slice bf16
  [4096, 512] per core.
Each core computes outT_c = wmT_c.T @ xT as [512, 8192] via bf16
matmuls (1 cycle/row, same PE rate as f32r but half the DMA traffic
and zero DVE rounding work); the host transposes/concatenates.

Per-core device kernel:
  - wmT resident in SBUF as 32 single tiles [128, 512] bf16 (4 MB).
  - batch chunk 0 is 1024 cols wide (8 psum banks, k-major): the wide
    chunk halves the DMA supply rate the early matmuls demand
    (~220 GB/s vs ~300 at 512 wide), so the PE can start ~9 us in
    (right after the ~5 us engine preamble) without starving mid-chunk.
  - chunks 1..13 are 512 wide: 8 sub-DMAs [128, 4, 512] bf16, 4 psum
    tiles [128 out, 512 b] each accumulating 32 matmuls (lhsT = wm
    slice [128 k, 128 out], rhs = x sub [128 k, 512 b]), k-major with
    next-chunk prefetch between matmuls and drain.
  - last chunk runs as 2 half-width (256 col) passes, oc-major, so
    the final psum drain + output DMA tail is halved.
  - psum -> SBUF f32 copy (DVE) -> DMA to outT.
PE does matmuls only; DMA (84 MB/core) runs at ~54% duty so the
kernel sits at the dense-GEMM compute roofline (~443 us span at
2.4 GHz; chip power states can derate the PE to ~2.0 GHz run-to-run).
"""

import ml_dtypes
import numpy as np

import concourse.bass as bass
import concourse.mybir as mybir
import concourse.tile as tile
from concourse import bacc
from concourse.bass_utils import run_bass_kernel_spmd

P = 128
D_IN = 4096
D_OUT = 4096
BATCH = 8192
N_CORES = 8
O_PER_CORE = D_OUT // N_CORES  # 512
KC = D_IN // P  # 32 contraction chunks of 128
B_CHUNK = 512
OT = O_PER_CORE // P  # 4 output partition tiles
KG = 8  # k groups per 512-wide chunk (sub-DMA granularity)
KCG = KC // KG  # 4 ks of 128 per group

W0 = 1024  # width of the first (wide) chunk
N_MID = (BATCH - W0 - B_CHUNK) // B_CHUNK  # 13 middle chunks

F32 = mybir.dt.float32
BF16 = mybir.dt.bfloat16


def build_nc():
    nc = bacc.Bacc("TRN2", target_bir_lowering=False, debug=False, num_devices=N_CORES)

    xT_d = nc.dram_tensor("xT", [D_IN, BATCH], BF16, kind="ExternalInput")
    wmT_d = nc.dram_tensor("wmT", [D_IN, O_PER_CORE], BF16, kind="ExternalInput")
    outT_d = nc.dram_tensor("outT", [O_PER_CORE, BATCH], F32, kind="ExternalOutput")

    with tile.TileContext(nc) as tc:
        with (
            tc.tile_pool(name="persist", bufs=1) as persist,
            tc.tile_pool(name="x0", bufs=16) as x0pool,
            tc.tile_pool(name="xs", bufs=16) as xspool,
            tc.tile_pool(name="outp", bufs=8) as outp,
            tc.tile_pool(name="mpsum", bufs=8, space="PSUM") as mpsum,
        ):
            # --- prologue: weights as 32 singles [128, 512], interleaved
            # with chunk 0's x pieces [128, 2, 1024] so MM(ic=0) waits for
            # only ~640 KB of DMA ---
            wm_t = []
            x0_t = []
            for ic in range(KC):
                wm = persist.tile([P, O_PER_CORE], BF16, name=f"wmT{ic}")
                nc.sync.dma_start(wm, wmT_d[ic * P : (ic + 1) * P, :])
                wm_t.append(wm)
                if ic % 2 == 0:
                    x0 = x0pool.tile([P, 2, W0], BF16, tag="x0", name=f"x0_{ic}")
                    rows = slice(ic * P, (ic + 2) * P)
                    nc.sync.dma_start(
                        x0, xT_d[rows, 0:W0].rearrange("(kc p) b -> p kc b", p=P)
                    )
                    x0_t.append(x0)

            def lhsT(ic, oc):
                return wm_t[ic][:, oc * P : (oc + 1) * P]

            def emit_x_sub(bc_col, g):
                xs = xspool.tile([P, KCG, B_CHUNK], BF16, tag="xs", name="xs")
                rows = slice(g * (D_IN // KG), (g + 1) * (D_IN // KG))
                cols = slice(bc_col, bc_col + B_CHUNK)
                nc.sync.dma_start(
                    xs, xT_d[rows, cols].rearrange("(kc p) b -> p kc b", p=P)
                )
                return xs

            def drain(psum, oc, bcol, ncol):
                ob = outp.tile([P, ncol], F32)
                nc.vector.tensor_copy(ob, psum)
                nc.sync.dma_start(
                    outT_d[oc * P : (oc + 1) * P, bcol : bcol + ncol], ob
                )

            # --- chunk 0: 1024 wide, 8 psums (oc x half), k-major ---
            psums = [
                [
                    mpsum.tile([P, B_CHUNK], F32, name=f"ps0_{oc}_{h}", tag="ps")
                    for h in range(2)
                ]
                for oc in range(OT)
            ]
            for ic in range(KC):
                for oc in range(OT):
                    for h in range(2):
                        nc.tensor.matmul(
                            psums[oc][h],
                            lhsT(ic, oc),
                            x0_t[ic // 2][:, ic % 2, h * B_CHUNK : (h + 1) * B_CHUNK],
                            start=(ic == 0),
                            stop=(ic == KC - 1),
                        )
            pending = [emit_x_sub(W0, g) for g in range(KG)]
            for oc in range(OT):
                for h in range(2):
                    drain(psums[oc][h], oc, h * B_CHUNK, B_CHUNK)

            # --- middle chunks: 512 wide, k-major so all 4 psums finish
            # together; prefetch next chunk before draining ---
            for mc in range(N_MID):
                col = W0 + mc * B_CHUNK
                x_subs = pending
                psums = [
                    mpsum.tile([P, B_CHUNK], F32, name=f"ps{oc}", tag="ps")
                    for oc in range(OT)
                ]
                for g in range(KG):
                    for k in range(KCG):
                        ic = g * KCG + k
                        for oc in range(OT):
                            nc.tensor.matmul(
                                psums[oc],
                                lhsT(ic, oc),
                                x_subs[g][:, k, :],
                                start=(ic == 0),
                                stop=(ic == KC - 1),
                            )
                pending = [emit_x_sub(col + B_CHUNK, g) for g in range(KG)]
                for oc in range(OT):
                    drain(psums[oc], oc, col, B_CHUNK)

            # --- last chunk: two half-width (256 col) passes, oc-major,
            # so each psum finishes early and the final drain tail is
            # half a tile ---
            x_subs = pending
            last_col = BATCH - B_CHUNK
            HB = B_CHUNK // 2
            for h in range(2):
                for oc in range(OT):
                    ps = mpsum.tile([P, HB], F32, name=f"psl{h}{oc}", tag="ps")
                    for g in range(KG):
                        for k in range(KCG):
                            ic = g * KCG + k
                            nc.tensor.matmul(
                                ps,
                                lhsT(ic, oc),
                                x_subs[g][:, k, h * HB : (h + 1) * HB],
                                start=(ic == 0),
                                stop=(ic == KC - 1),
                            )
                    drain(ps, oc, last_col + h * HB, HB)

    nc.compile()
    return nc


_NC_CACHE = None


def _shard_inputs(x, weight, mask):
    """Host-side marshalling: mask-multiply, transpose, bf16, per-core slice."""
    x = np.asarray(x, dtype=np.float32)
    weight = np.asarray(weight, dtype=np.float32)
    mask = np.asarray(mask, dtype=np.float32)
    xT = np.ascontiguousarray(x.T).astype(ml_dtypes.bfloat16)
    wmT = (weight * mask).T  # [D_IN, D_OUT]
    in_maps = []
    for c in range(N_CORES):
        sl = slice(c * O_PER_CORE, (c + 1) * O_PER_CORE)
        in_maps.append(
            {
                "xT": xT,
                "wmT": np.ascontiguousarray(wmT[:, sl]).astype(ml_dtypes.bfloat16),
            }
        )
    return in_maps


def kernel(x, weight, mask):
    global _NC_CACHE
    if _NC_CACHE is None:
        _NC_CACHE = build_nc()
    nc = _NC_CACHE

    in_maps = _shard_inputs(x, weight, mask)
    res = run_bass_kernel_spmd(nc, in_maps, core_ids=list(range(N_CORES)))

    out = np.empty((BATCH, D_OUT), dtype=np.float32)
    for c in range(N_CORES):
        sl = slice(c * O_PER_CORE, (c + 1) * O_PER_CORE)
        out[:, sl] = res.results[c]["outT"].T
    return out
